# revision 19
# baseline (speedup 1.0000x reference)
"""Trainium2 Bass kernel for nn_LogLinearAttention.

Math: the reference computes
    q = x@Wq.T+bq ; v = x@Wv.T+bv ; r = x@Wr.T+br
    scores = q @ v.T ; attn = softmax(scores, axis=1)   # over the QUERY axis
    emb[b,s,:] = sum_t attn[b,s,t] r[b,t,:] ; pooled = emb.sum(axis=1)
    out = sigmoid(pooled @ Wl.T + bl)

Because softmax normalizes over axis 1 and pooled sums over that same
axis, sum_s attn[s, t] == 1 for every t, so
    pooled[b] = sum_t r[b, t, :] = (sum_t x[b, t, :]) @ Wr.T + S*br
and the q/v projections and the S x S attention cancel exactly:
    out[b] = sigmoid( xsum[b] . (Wl@Wr) + S*(br . Wl) + bl )

The kernel therefore only needs a sequence-sum of x (the only large
input, 32MB total) plus tiny weight contractions. Data-parallel over
batch: core b handles x[b] (4MB), weights replicated.

Per-core device program (v4 — all fp32, HWDGE DMAs only):
  - x[b] arrives as 16 slice DMAs of [128,512] (256KB each), split
    across the sync and scalar HWDGE rings, all issued up-front
    (bufs=16) so arrival is continuous from ~8us.
  - acc[128,512] += slice on the vector engine as each slice lands; the
    DVE stream carries NOTHING but these adds until the tail, so a
    late-arriving weight DMA can never stall the chain.
  - All weights (Wr+Wl+br+bl) pack into ONE [128,2057] DMA (every DMA
    completion pays a ~3us straggler-engine lag, so count is minimized).
  - w_rep[128,512] = broadcast(Wl@Wr) on the TensorEngine via a
    free-dim-broadcast stationary operand; runs mid-stream.
  - The bias constant S*(br.Wl)+bl is computed entirely on GpSimd
    (tensor ops + XYZWC reduce), keeping DVE and PE clear.
  - tail: acc *= w_rep ; row-reduce ; 128->1 matmul with ones ;
    sigmoid (table prewarmed at kernel start) ; DMA the [1,1] out.
"""

import numpy as np

B, S, D = 8, 2048, 512
P = 128
NSL = 16  # x slice DMAs per core (256KB each)
JW = 4  # Wr/Wl/br rows per partition
ESH = D // B  # Wr column-shard width per core (AllGather reassembles)
WCOL = JW * ESH + 9  # packed weight columns: Wr shard | wl | br | bl pad
N_SYNC = 10  # slices on the sync ring; rest go on the scalar ring
# ring loads: sync 10 x 256KB = 2.5MB ; scalar = wp (1MB) + 6 x 256KB = 2.5MB

_CACHE = {}


def _build():
    import concourse.bacc as bacc
    import concourse.mybir as mybir
    import concourse.tile as tile

    f32 = mybir.dt.float32

    nc = bacc.Bacc(
        "TRN2",
        target_bir_lowering=False,
        debug=False,
        enable_asserts=False,
        num_devices=B,
    )
    x_d = nc.dram_tensor("x", [NSL, P, D], f32, kind="ExternalInput").ap()
    wp_d = nc.dram_tensor("wp", [P, WCOL], f32, kind="ExternalInput").ap()
    cc_in_d = nc.dram_tensor("cc_in", [P, ESH], f32).ap()
    cc_out_d = nc.dram_tensor("cc_out", [B, P, ESH], f32, addr_space="Shared").ap()
    out_d = nc.dram_tensor("out", [1, 1], f32, kind="ExternalOutput").ap()

    with tile.TileContext(nc) as tc:
        with (
            tc.tile_pool(name="xp", bufs=NSL) as xp,
            tc.tile_pool(name="sg", bufs=1) as sg,
            tc.tile_pool(name="ps", bufs=1, space="PSUM") as ps,
        ):
            ones = sg.tile([P, 1], f32, tag="ones")
            nc.vector.memset(ones, 1.0)
            # Prewarm the sigmoid activation table (1.3us) off the
            # critical path: a dummy [1,1] sigmoid right at the start.
            warm = sg.tile([1, 1], f32, tag="warm")
            nc.scalar.activation(
                warm, ones[0:1, 0:1], mybir.ActivationFunctionType.Sigmoid
            )

            # One DMA for every weight byte, first on the scalar ring.
            wp = sg.tile([P, WCOL], f32, tag="wp")
            nc.scalar.dma_start(wp, wp_d)
            wt = wp[:, : JW * ESH].rearrange("p (j e) -> p j e", j=JW)
            wlt = wp[:, JW * ESH : JW * ESH + JW]
            brt = wp[:, JW * ESH + JW : JW * ESH + 2 * JW]
            blt = wp[0:1, JW * ESH + 2 * JW : JW * ESH + 2 * JW + 1]

            # acc[128, D] accumulates the x stream on the vector engine.
            # One DMA + one add per 256KB slice; nothing else ever enters
            # the DVE stream before the tail.
            acc = sg.tile([P, D], f32, tag="acc")
            xts = []
            for n in range(NSL):
                xt = xp.tile([P, D], f32, tag="xt")
                eng = nc.sync if n < N_SYNC else nc.scalar
                eng.dma_start(xt, x_d[n])
                xts.append(xt)
                if n == 1:
                    nc.vector.tensor_add(out=acc, in0=xts[0], in1=xts[1])
                elif n > 1:
                    nc.vector.tensor_add(out=acc, in0=acc, in1=xt)

            # This core's 64-wide shard of w = Wl @ Wr, broadcast over
            # partitions: lhsT[k, p] = Wl[4k+j] for all p via a free-dim
            # broadcast of the [128,1] Wl column; rhs = Wr shard rows.
            wrep_ps = ps.tile([P, ESH], f32, tag="wrep")
            for j in range(JW):
                nc.tensor.matmul(
                    wrep_ps,
                    wlt[:, j : j + 1].to_broadcast([P, P]),
                    wt[:, j, :],
                    start=(j == 0),
                    stop=(j == JW - 1),
                )
            # AllGather the 8 shards into the full [128, D] broadcast w.
            # All hops ride GpSimd/SWDGE + the CC queue, far off the
            # critical path (done mid-stream).
            cc_in_sb = sg.tile([P, ESH], f32, tag="cc_in")
            nc.scalar.activation(
                cc_in_sb, wrep_ps, mybir.ActivationFunctionType.Copy
            )
            nc.gpsimd.dma_start(cc_in_d, cc_in_sb)
            nc.gpsimd.collective_compute(
                "AllGather",
                mybir.AluOpType.bypass,
                replica_groups=[list(range(B))],
                ins=[cc_in_d],
                outs=[cc_out_d],
            )
            w_rep3 = sg.tile([P, B, ESH], f32, tag="w_rep")
            nc.gpsimd.dma_start(w_rep3, cc_out_d.rearrange("r p e -> p r e"))
            w_rep = w_rep3.rearrange("p r e -> p (r e)")

            # constant term on GpSimd: t2 = S * dot(br, Wl) + bl
            prod2 = sg.tile([P, JW], f32, tag="prod2")
            nc.gpsimd.tensor_mul(out=prod2, in0=brt, in1=wlt)
            c_sb = sg.tile([1, 1], f32, tag="c_sb")
            nc.gpsimd.tensor_reduce(
                c_sb, prod2, axis=mybir.AxisListType.XYZWC, op=mybir.AluOpType.add
            )
            t2 = sg.tile([1, 1], f32, tag="t2")
            nc.gpsimd.tensor_scalar_mul(t2, c_sb, float(S))
            nc.gpsimd.tensor_add(out=t2, in0=t2, in1=blt)

            # tail: logit = sum_{p,d} acc*w_rep + t2 ; sigmoid.
            # (tensor_tensor_reduce would fuse these two DVE passes but
            # crashes the NEFF at execute time on this toolchain.)
            nc.vector.tensor_mul(out=acc, in0=acc, in1=w_rep)
            red = sg.tile([P, 1], f32, tag="red")
            nc.vector.reduce_sum(red, acc, axis=mybir.AxisListType.X)
            c2_ps = ps.tile([1, 1], f32, tag="c2")
            nc.tensor.matmul(c2_ps, red, ones, start=True, stop=True)
            fin = sg.tile([1, 1], f32, tag="fin")
            nc.scalar.activation(
                fin,
                c2_ps,
                mybir.ActivationFunctionType.Sigmoid,
                bias=t2,
                scale=1.0,
            )
            # out goes on the scalar ring: the sync ring is still
            # retiring the last x slices when fin is ready.
            nc.scalar.dma_start(out_d, fin)

    nc.compile()
    return nc


def _in_maps(inputs):
    x = np.ascontiguousarray(np.asarray(inputs["x"], dtype=np.float32))
    Wr = np.asarray(inputs["Wr"], dtype=np.float32)
    br = np.asarray(inputs["br"], dtype=np.float32)
    Wl = np.asarray(inputs["Wl"], dtype=np.float32)
    bl = np.asarray(inputs["bl"], dtype=np.float32)

    wr3 = Wr.reshape(P, JW, D)
    maps = []
    for b in range(B):
        wp = np.zeros((P, WCOL), dtype=np.float32)
        wp[:, : JW * ESH] = wr3[:, :, b * ESH : (b + 1) * ESH].reshape(P, JW * ESH)
        wp[:, JW * ESH : JW * ESH + JW] = Wl.reshape(P, JW)
        wp[:, JW * ESH + JW : JW * ESH + 2 * JW] = br.reshape(P, JW)
        wp[0, JW * ESH + 2 * JW] = bl[0]
        maps.append({"x": x[b].reshape(NSL, P, D), "wp": wp})
    return maps


def get_nc():
    if "nc" not in _CACHE:
        _CACHE["nc"] = _build()
    return _CACHE["nc"]


def kernel(**inputs) -> np.ndarray:
    from concourse.bass_utils import run_bass_kernel_spmd

    nc = get_nc()
    res = run_bass_kernel_spmd(nc, _in_maps(inputs), list(range(B)))
    out = np.stack([res.results[b]["out"].reshape(()) for b in range(B)])
    return out.reshape(B, 1).astype(np.float32)
